# revision 16
# baseline (speedup 1.0000x reference)
"""GroupedQueryAttention Trainium2 Bass kernel.

Problem: B=2, S=2048, D_IN=2048, 32 Q heads / 8 KV groups, head_dim=64.
Sharding: data-parallel over batch (2) x tensor-parallel over KV groups (4):
core c -> batch c//4, group-pair c%4 (2 KV groups = 8 Q heads per core).
Each core computes its heads' attention plus a partial output projection;
the host sums the 4 partial outputs per batch.

Per-core kernel design (all matmuls bf16 with fp32 PSUM accumulation):
 - Host passes x^T and transposed weight slices so every matmul contracts
   over the partition dim with no on-device transposes of x or W.
 - Scores are computed transposed, S^T[k,q], so softmax exp() output IS
   P^T (the layout the PV matmul needs) - no P transposes on any engine.
 - No max-subtraction in softmax (scores are O(3), exp is safe in fp32;
   mathematically identical to the reference softmax).
 - The softmax denominator rides along as a 65th "ones" column of V:
   ctx_aug^T = V_aug^T @ P^T gives ctx^T rows 0..63 and the denominator
   in row 64 of the same PSUM accumulation.
 - kt blocks processed in pairs sharing one [128,1024] PSUM tile and one
   exp() to amortize the ACT fixed overhead; causal mask = bf16 0/1
   multiply on GpSimd for the two diagonal pairs only.
 - Per-tb/per-qb tiles keep cross-phase dependencies block-granular; the
   output projection for q-block qb is emitted inside attention of qb+1
   to fill TensorE gaps while ACT runs exp.
"""

import numpy as np
import ml_dtypes

B, S, D = 2, 2048, 2048
HD = 64

_NC = None
_DEBUG = False


def _build():
    import concourse.mybir as mybir
    import concourse.tile as tile
    from concourse import bacc

    bf16 = mybir.dt.bfloat16
    f32 = mybir.dt.float32
    EXP = mybir.ActivationFunctionType.Exp
    MUL = mybir.AluOpType.mult

    nc = bacc.Bacc("TRN2", target_bir_lowering=False, debug=False, num_devices=8)

    d_xt = nc.dram_tensor("xt", [16, 128, S], bf16, kind="ExternalInput")
    d_wqt = nc.dram_tensor("wqt", [16, 128, 512], bf16, kind="ExternalInput")
    d_wkt = nc.dram_tensor("wkt", [16, 128, 128], bf16, kind="ExternalInput")
    d_wvt = nc.dram_tensor("wvt", [16, 128, 128], bf16, kind="ExternalInput")
    d_wot = nc.dram_tensor("wot", [4, 128, D], bf16, kind="ExternalInput")
    d_mask2 = nc.dram_tensor("mask2", [128, 896], f32, kind="ExternalInput")

    d_out = nc.dram_tensor("out", [S, D], f32, kind="ExternalOutput")
    if _DEBUG:
        d_dq = nc.dram_tensor("dq", [64, 8, 512], bf16, kind="ExternalOutput")
        d_dk = nc.dram_tensor("dk", [64, 2, 512], bf16, kind="ExternalOutput")
        d_dv = nc.dram_tensor("dv", [128, 4, 2, 65], bf16, kind="ExternalOutput")
        d_dc = nc.dram_tensor("dc", [128, 4, 512], bf16, kind="ExternalOutput")
    d_kc = nc.dram_tensor("kc", [2, 64, S], f32, kind="ExternalOutput")
    d_vc = nc.dram_tensor("vc", [S, 128], f32, kind="ExternalOutput")

    with tile.TileContext(nc) as tc:
        with (
            tc.tile_pool(name="cst", bufs=1) as cst,
            tc.tile_pool(name="xtp", bufs=32) as xtp,
            tc.tile_pool(name="wk", bufs=4) as wk,
            tc.tile_pool(name="ptp", bufs=6) as ptpool,
            tc.tile_pool(name="ps", bufs=2, space="PSUM") as psp,
            tc.tile_pool(name="psc", bufs=3, space="PSUM") as psc,
            tc.tile_pool(name="pso", bufs=1, space="PSUM") as pso,
        ):
            # weights needed first go first; wot (phase 3 only) last
            wkt = cst.tile([128, 16, 128], bf16, tag="wkt")
            nc.sync.dma_start(wkt[:], d_wkt.ap().rearrange("i p f -> p i f"))
            wvt = cst.tile([128, 16, 128], bf16, tag="wvt")
            nc.sync.dma_start(wvt[:], d_wvt.ap().rearrange("i p f -> p i f"))
            wqt = cst.tile([128, 16, 512], bf16, tag="wqt")
            nc.sync.dma_start(wqt[:], d_wqt.ap().rearrange("i p f -> p i f"))
            mask2 = cst.tile([128, 896], f32, tag="mask2")
            wot = cst.tile([128, 4, D], bf16, tag="wot")

            # per-block persistent activations
            qt_tb = [cst.tile([64, 8, 512], bf16, tag=f"qt{tb}", name=f"qt{tb}") for tb in range(4)]
            kt_tb = [cst.tile([64, 2, 512], bf16, tag=f"kt{tb}", name=f"kt{tb}") for tb in range(4)]
            va_tb = [cst.tile([128, 4, 2, 65], bf16, tag=f"va{tb}", name=f"va{tb}") for tb in range(4)]
            cx_qb = [cst.tile([128, 4, 512], bf16, tag=f"cx{qb}", name=f"cx{qb}") for qb in range(4)]
            for tb in range(4):
                nc.vector.memset(va_tb[tb][:, :, :, 64:65], 1.0)

            # ---------------- Phase 1: projections ----------------
            for tb in range(4):
                if tb == 1:
                    # deferred input DMAs: not needed until attention/outproj
                    nc.sync.dma_start(mask2[:], d_mask2.ap())
                    nc.sync.dma_start(wot[:], d_wot.ap().rearrange("c p o -> p c o"))
                ts_ = slice(tb * 512, (tb + 1) * 512)
                xts = []
                for it in range(16):
                    xt = xtp.tile([128, 512], bf16, tag="xt")
                    nc.sync.dma_start(xt[:], d_xt.ap()[it, :, ts_])
                    xts.append(xt)

                # K projection: both groups in one 128-wide output
                kp = psp.tile([128, 1024], f32, tag="ps")
                for it in range(16):
                    nc.tensor.matmul(
                        kp[:, :512], wkt[:, it, :], xts[it][:],
                        start=(it == 0), stop=(it == 15),
                    )
                kstg = wk.tile([128, 512], bf16, tag="kstg")
                nc.any.tensor_copy(out=kstg[:], in_=kp[:, :512])
                for g in range(2):
                    nc.sync.dma_start(
                        kt_tb[tb][:, g, :], kstg[g * 64:(g + 1) * 64, :]
                    )
                kf = wk.tile([128, 512], f32, tag="kf")
                nc.any.tensor_copy(out=kf[:], in_=kp[:, :512])
                for g in range(2):
                    nc.sync.dma_start(
                        d_kc.ap()[g, :, ts_], kf[g * 64:(g + 1) * 64, :]
                    )

                # Q projection: head pairs, full 128-wide outputs
                for pj in range(4):
                    qp = psp.tile([128, 1024], f32, tag="ps")
                    for it in range(16):
                        nc.tensor.matmul(
                            qp[:, :512], wqt[:, it, pj * 128:(pj + 1) * 128],
                            xts[it][:], start=(it == 0), stop=(it == 15),
                        )
                    qstg = wk.tile([128, 512], bf16, tag="qstg")
                    nc.any.tensor_copy(out=qstg[:], in_=qp[:, :512])
                    for j in range(2):
                        nc.sync.dma_start(
                            qt_tb[tb][:, 2 * pj + j, :],
                            qstg[j * 64:(j + 1) * 64, :],
                        )

                # V projection: token-major tiles
                for tl in range(4):
                    vp = psp.tile([128, 1024], f32, tag="ps")
                    for it in range(16):
                        nc.tensor.matmul(
                            vp[:, :128], xts[it][:, tl * 128:(tl + 1) * 128],
                            wvt[:, it, :], start=(it == 0), stop=(it == 15),
                        )
                    for g in range(2):
                        nc.any.tensor_copy(
                            out=va_tb[tb][:, tl, g, 0:64],
                            in_=vp[:, g * 64:(g + 1) * 64],
                        )
                    vf = wk.tile([128, 128], f32, tag="vf")
                    nc.any.tensor_copy(out=vf[:], in_=vp[:, :128])
                    tt = tb * 4 + tl
                    nc.sync.dma_start(d_vc.ap()[tt * 128:(tt + 1) * 128, :], vf[:])

            # ------------- Phase 2 + interleaved output projection -------------
            def outproj_tile(tt, ob):
                qb = tt // 4
                tsl = slice((tt % 4) * 128, (tt % 4 + 1) * 128)
                osl = slice(ob * 512, (ob + 1) * 512)
                op = pso.tile([128, 512], f32, tag="pso")
                for ct in range(4):
                    nc.tensor.matmul(
                        op[:], cx_qb[qb][:, ct, tsl], wot[:, ct, osl],
                        start=(ct == 0), stop=(ct == 3),
                    )
                ot = wk.tile([128, 512], f32, tag="ot")
                nc.vector.tensor_copy(out=ot[:], in_=op[:])
                tgl = slice((tt) * 128, (tt + 1) * 128)
                nc.sync.dma_start(d_out.ap()[tgl, osl], ot[:])

            pending_tails = []

            def normalize_tail(cp, h, qb):
                # row 64 of cp is the softmax denominator
                rr = wk.tile([1, 512], f32, tag="rr", name="rr")
                nc.vector.reciprocal(rr[:], cp[64:65, :])
                rbs = wk.tile([64, 512], f32, tag="rbs", name="rbs")
                nc.gpsimd.partition_broadcast(rbs[:], rr[:])
                stg = wk.tile([64, 512], bf16, tag="stg", name="stg")
                nc.vector.tensor_tensor(stg[:], cp[0:64, :], rbs[:], MUL)
                poff = (h % 2) * 64
                nc.sync.dma_start(cx_qb[qb][poff:poff + 64, h // 2, :], stg[:])

            for qb in range(4):
                npair = 2 * (qb + 1)
                for hidx in range(8):
                    g, r4 = divmod(hidx, 4)
                    h = hidx
                    cp = psc.tile([65, 512], f32, tag="psc")
                    pend = []

                    def flush_pv(cp=cp, npair=npair, g=g, pend=pend):
                        pj, ptp = pend.pop(0)
                        for u in range(2):
                            kt = 2 * pj + u
                            nc.tensor.matmul(
                                cp[:], va_tb[kt // 4][:, kt % 4, g, :],
                                ptp[:, u * 512:(u + 1) * 512],
                                start=(kt == 0), stop=(kt == 2 * npair - 1),
                            )

                    for pj in range(npair):
                        sp = psp.tile([128, 1024], f32, tag="ps")
                        for u in range(2):
                            kt = 2 * pj + u
                            nc.tensor.matmul(
                                sp[:, u * 512:(u + 1) * 512],
                                kt_tb[kt // 4][:, g, (kt % 4) * 128:(kt % 4 + 1) * 128],
                                qt_tb[qb][:, h, :],
                                start=True, stop=True,
                            )
                        if pj >= npair - 2:
                            for u in range(2):
                                kt_loc = 2 * (pj - (npair - 2)) + u
                                off = 128 * kt_loc
                                w = min(512, 128 + off)
                                s0 = 384 - off
                                nc.vector.tensor_tensor(
                                    sp[:, u * 512:u * 512 + w], sp[:, u * 512:u * 512 + w],
                                    mask2[:, s0:s0 + w],
                                    mybir.AluOpType.add,
                                )
                        ptp = ptpool.tile([128, 1024], bf16, tag="pt")
                        nc.scalar.activation(ptp[:], sp[:], EXP)
                        pend.append((pj, ptp))
                        if len(pend) >= 3:
                            flush_pv()
                    while pend:
                        flush_pv()

                    # deferred normalize of the previous head: its GpSimd
                    # broadcast must queue after this head's masks so an
                    # in-order GpSimd never blocks masks behind it
                    if pending_tails:
                        normalize_tail(*pending_tails.pop(0))
                    pending_tails.append((cp, h, qb))

                    # interleave previous q-block's output projection
                    if qb >= 1 and hidx < 8:
                        tt = (qb - 1) * 4 + hidx // 2
                        ob = (hidx % 2) * 2
                        outproj_tile(tt, ob)
                        outproj_tile(tt, ob + 1)

            for t_ in pending_tails:
                normalize_tail(*t_)

            if _DEBUG:
                nc.sync.dma_start(d_dq.ap()[:], qt_tb[0][:])
                nc.sync.dma_start(d_dk.ap()[:], kt_tb[0][:])
                nc.sync.dma_start(d_dv.ap()[:], va_tb[0][:])
                nc.sync.dma_start(d_dc.ap()[:], cx_qb[0][:])

            for hidx in range(8):
                tt = 12 + hidx // 2
                ob = (hidx % 2) * 2
                outproj_tile(tt, ob)
                outproj_tile(tt, ob + 1)

    nc.compile()
    return nc


def _get_nc():
    global _NC
    if _NC is None:
        _NC = _build()
    return _NC


def _prep_inputs(x, Wq, Wk, Wv, Wo):
    bf = ml_dtypes.bfloat16
    xt_b = []
    for b in range(B):
        xt = np.ascontiguousarray(x[b].T).astype(bf)  # [D, S]
        xt_b.append(xt.reshape(16, 128, S))
    # master causal additive mask: valid iff p <= j - 384
    p = np.arange(128)[:, None]
    j = np.arange(896)[None, :]
    mask2 = np.where(p <= j - 384, 0.0, -1e30).astype(np.float32)  # [128, 896]

    in_maps = []
    for c in range(8):
        b, tp = divmod(c, 4)
        wq_s = (0.125 * Wq[tp * 512:(tp + 1) * 512]).astype(np.float32)  # [512, D]
        wqt = np.ascontiguousarray(wq_s.T).astype(bf).reshape(16, 128, 512)
        wkt = np.ascontiguousarray(Wk[tp * 128:(tp + 1) * 128].T).astype(bf).reshape(16, 128, 128)
        wvt = np.ascontiguousarray(Wv[tp * 128:(tp + 1) * 128].T).astype(bf).reshape(16, 128, 128)
        wot = np.ascontiguousarray(Wo[:, tp * 512:(tp + 1) * 512].T).astype(bf).reshape(4, 128, D)
        in_maps.append({
            "xt": xt_b[b], "wqt": wqt, "wkt": wkt, "wvt": wvt, "wot": wot,
            "mask2": mask2,
        })
    return in_maps


def _run(x, Wq, Wk, Wv, Wo, trace=False, trace_cores=None):
    from concourse.bass_utils import run_bass_kernel_spmd

    nc = _get_nc()
    in_maps = _prep_inputs(x, Wq, Wk, Wv, Wo)
    kwargs = {}
    if trace:
        kwargs = dict(trace=True, trace_cores=trace_cores or [0])
    res = run_bass_kernel_spmd(nc, in_maps, core_ids=list(range(8)), **kwargs)

    out = np.zeros((B, S, D), dtype=np.float32)
    keys = np.zeros((B, 8, S, HD), dtype=np.float32)
    values = np.zeros((B, 8, S, HD), dtype=np.float32)
    for c in range(8):
        b, tp = divmod(c, 4)
        r = res.results[c]
        out[b] += r["out"]
        for g in range(2):
            keys[b, 2 * tp + g] = r["kc"][g].T
            values[b, 2 * tp + g] = r["vc"][:, g * 64:(g + 1) * 64]
    return (out, (keys, values)), res


def kernel(x, Wq, Wk, Wv, Wo):
    outputs, _ = _run(np.asarray(x), np.asarray(Wq), np.asarray(Wk),
                      np.asarray(Wv), np.asarray(Wo))
    return outputs


# revision 17
# speedup vs baseline: 1.1641x; 1.1641x over previous
"""GroupedQueryAttention Trainium2 Bass kernel.

Problem: B=2, S=2048, D_IN=2048, 32 Q heads / 8 KV groups, head_dim=64.
Sharding: data-parallel over batch (2) x tensor-parallel over KV groups (4):
core c -> batch c//4, group-pair c%4 (2 KV groups = 8 Q heads per core).
Each core computes its heads' attention plus a partial output projection;
the host sums the 4 partial outputs per batch.

Per-core kernel design (all matmuls bf16 with fp32 PSUM accumulation):
 - Host passes x^T and transposed weight slices so every matmul contracts
   over the partition dim with no on-device transposes of x or W.
 - Scores are computed transposed, S^T[k,q], so softmax exp() output IS
   P^T (the layout the PV matmul needs) - no P transposes on any engine.
 - No max-subtraction in softmax (scores are O(3), exp is safe in fp32;
   mathematically identical to the reference softmax).
 - The softmax denominator rides along as a 65th "ones" column of V:
   ctx_aug^T = V_aug^T @ P^T gives ctx^T rows 0..63 and the denominator
   in row 64 of the same PSUM accumulation.
 - kt blocks processed in pairs sharing one [128,1024] PSUM tile and one
   exp() to amortize the ACT fixed overhead; causal mask = bf16 0/1
   multiply on GpSimd for the two diagonal pairs only.
 - Per-tb/per-qb tiles keep cross-phase dependencies block-granular; the
   output projection for q-block qb is emitted inside attention of qb+1
   to fill TensorE gaps while ACT runs exp.
"""

import numpy as np
import ml_dtypes

B, S, D = 2, 2048, 2048
HD = 64

_NC = None
_DEBUG = False


def _build():
    import concourse.mybir as mybir
    import concourse.tile as tile
    from concourse import bacc

    bf16 = mybir.dt.bfloat16
    f32 = mybir.dt.float32
    EXP = mybir.ActivationFunctionType.Exp
    MUL = mybir.AluOpType.mult

    nc = bacc.Bacc("TRN2", target_bir_lowering=False, debug=False, num_devices=8)

    d_xt = nc.dram_tensor("xt", [16, 128, S], bf16, kind="ExternalInput")
    d_wqt = nc.dram_tensor("wqt", [16, 128, 512], bf16, kind="ExternalInput")
    d_wkt = nc.dram_tensor("wkt", [16, 128, 128], bf16, kind="ExternalInput")
    d_wvt = nc.dram_tensor("wvt", [16, 128, 128], bf16, kind="ExternalInput")
    d_wot = nc.dram_tensor("wot", [4, 128, D], bf16, kind="ExternalInput")
    d_mask2 = nc.dram_tensor("mask2", [128, 896], f32, kind="ExternalInput")

    d_out = nc.dram_tensor("out", [S, D], f32, kind="ExternalOutput")
    if _DEBUG:
        d_dq = nc.dram_tensor("dq", [64, 8, 512], bf16, kind="ExternalOutput")
        d_dk = nc.dram_tensor("dk", [64, 2, 512], bf16, kind="ExternalOutput")
        d_dv = nc.dram_tensor("dv", [128, 4, 2, 65], bf16, kind="ExternalOutput")
        d_dc = nc.dram_tensor("dc", [128, 4, 512], bf16, kind="ExternalOutput")
    d_kc = nc.dram_tensor("kc", [2, 64, S], f32, kind="ExternalOutput")
    d_vc = nc.dram_tensor("vc", [S, 128], f32, kind="ExternalOutput")

    with tile.TileContext(nc) as tc:
        with (
            tc.tile_pool(name="cst", bufs=1) as cst,
            tc.tile_pool(name="xtp", bufs=32) as xtp,
            tc.tile_pool(name="wk", bufs=4) as wk,
            tc.tile_pool(name="ptp", bufs=6) as ptpool,
            tc.tile_pool(name="ps", bufs=2, space="PSUM") as psp,
            tc.tile_pool(name="psc", bufs=2, space="PSUM") as psc,
            tc.tile_pool(name="pso", bufs=2, space="PSUM") as pso,
        ):
            # weights needed first go first; wot (phase 3 only) last
            wkt = cst.tile([128, 16, 128], bf16, tag="wkt")
            nc.sync.dma_start(wkt[:], d_wkt.ap().rearrange("i p f -> p i f"))
            wvt = cst.tile([128, 16, 128], bf16, tag="wvt")
            nc.sync.dma_start(wvt[:], d_wvt.ap().rearrange("i p f -> p i f"))
            wqt = cst.tile([128, 16, 512], bf16, tag="wqt")
            nc.sync.dma_start(wqt[:], d_wqt.ap().rearrange("i p f -> p i f"))
            mask2 = cst.tile([128, 896], f32, tag="mask2")
            wot = cst.tile([128, 4, D], bf16, tag="wot")

            # per-block persistent activations
            qt_tb = [cst.tile([64, 8, 512], bf16, tag=f"qt{tb}", name=f"qt{tb}") for tb in range(4)]
            kt_tb = [cst.tile([64, 2, 512], bf16, tag=f"kt{tb}", name=f"kt{tb}") for tb in range(4)]
            va_tb = [cst.tile([128, 4, 2, 65], bf16, tag=f"va{tb}", name=f"va{tb}") for tb in range(4)]
            cx_qb = [cst.tile([128, 4, 512], bf16, tag=f"cx{qb}", name=f"cx{qb}") for qb in range(4)]
            for tb in range(4):
                nc.vector.memset(va_tb[tb][:, :, :, 64:65], 1.0)

            # ---------------- Phase 1: projections ----------------
            for tb in range(4):
                if tb == 1:
                    # deferred input DMAs: not needed until attention/outproj
                    nc.sync.dma_start(mask2[:], d_mask2.ap())
                    nc.sync.dma_start(wot[:], d_wot.ap().rearrange("c p o -> p c o"))
                ts_ = slice(tb * 512, (tb + 1) * 512)
                xts = []
                for it in range(16):
                    xt = xtp.tile([128, 512], bf16, tag="xt")
                    nc.sync.dma_start(xt[:], d_xt.ap()[it, :, ts_])
                    xts.append(xt)

                # K projection: both groups in one 128-wide output
                kp = psp.tile([128, 1024], f32, tag="ps")
                for it in range(16):
                    nc.tensor.matmul(
                        kp[:, :512], wkt[:, it, :], xts[it][:],
                        start=(it == 0), stop=(it == 15),
                    )
                kstg = wk.tile([128, 512], bf16, tag="kstg")
                nc.any.tensor_copy(out=kstg[:], in_=kp[:, :512])
                for g in range(2):
                    nc.sync.dma_start(
                        kt_tb[tb][:, g, :], kstg[g * 64:(g + 1) * 64, :]
                    )
                kf = wk.tile([128, 512], f32, tag="kf")
                nc.any.tensor_copy(out=kf[:], in_=kp[:, :512])
                for g in range(2):
                    nc.sync.dma_start(
                        d_kc.ap()[g, :, ts_], kf[g * 64:(g + 1) * 64, :]
                    )

                # Q projection: head pairs, full 128-wide outputs
                for pj in range(4):
                    qp = psp.tile([128, 1024], f32, tag="ps")
                    for it in range(16):
                        nc.tensor.matmul(
                            qp[:, :512], wqt[:, it, pj * 128:(pj + 1) * 128],
                            xts[it][:], start=(it == 0), stop=(it == 15),
                        )
                    qstg = wk.tile([128, 512], bf16, tag="qstg")
                    nc.any.tensor_copy(out=qstg[:], in_=qp[:, :512])
                    for j in range(2):
                        nc.sync.dma_start(
                            qt_tb[tb][:, 2 * pj + j, :],
                            qstg[j * 64:(j + 1) * 64, :],
                        )

                # V projection: token-major tiles
                for tl in range(4):
                    vp = psp.tile([128, 1024], f32, tag="ps")
                    for it in range(16):
                        nc.tensor.matmul(
                            vp[:, :128], xts[it][:, tl * 128:(tl + 1) * 128],
                            wvt[:, it, :], start=(it == 0), stop=(it == 15),
                        )
                    for g in range(2):
                        nc.any.tensor_copy(
                            out=va_tb[tb][:, tl, g, 0:64],
                            in_=vp[:, g * 64:(g + 1) * 64],
                        )
                    vf = wk.tile([128, 128], f32, tag="vf")
                    nc.any.tensor_copy(out=vf[:], in_=vp[:, :128])
                    tt = tb * 4 + tl
                    nc.sync.dma_start(d_vc.ap()[tt * 128:(tt + 1) * 128, :], vf[:])

            # ------------- Phase 2 + interleaved output projection -------------
            def outproj_tile(tt, ob):
                qb = tt // 4
                tsl = slice((tt % 4) * 128, (tt % 4 + 1) * 128)
                osl = slice(ob * 512, (ob + 1) * 512)
                op = pso.tile([128, 512], f32, tag="pso")
                for ct in range(4):
                    nc.tensor.matmul(
                        op[:], cx_qb[qb][:, ct, tsl], wot[:, ct, osl],
                        start=(ct == 0), stop=(ct == 3),
                    )
                ot = wk.tile([128, 512], f32, tag="ot")
                nc.vector.tensor_copy(out=ot[:], in_=op[:])
                tgl = slice((tt) * 128, (tt + 1) * 128)
                nc.sync.dma_start(d_out.ap()[tgl, osl], ot[:])

            pending_tails = []

            def normalize_tail(cp, h, qb):
                # row 64 of cp is the softmax denominator
                rr = wk.tile([1, 512], f32, tag="rr", name="rr")
                nc.vector.reciprocal(rr[:], cp[64:65, :])
                rbs = wk.tile([64, 512], f32, tag="rbs", name="rbs")
                nc.gpsimd.partition_broadcast(rbs[:], rr[:])
                stg = wk.tile([64, 512], bf16, tag="stg", name="stg")
                nc.vector.tensor_tensor(stg[:], cp[0:64, :], rbs[:], MUL)
                poff = (h % 2) * 64
                nc.sync.dma_start(cx_qb[qb][poff:poff + 64, h // 2, :], stg[:])

            for qb in range(4):
                npair = 2 * (qb + 1)
                for hidx in range(8):
                    g, r4 = divmod(hidx, 4)
                    h = hidx
                    cp = psc.tile([65, 512], f32, tag="psc")
                    pend = []

                    def flush_pv(cp=cp, npair=npair, g=g, pend=pend):
                        pj, ptp = pend.pop(0)
                        for u in range(2):
                            kt = 2 * pj + u
                            nc.tensor.matmul(
                                cp[:], va_tb[kt // 4][:, kt % 4, g, :],
                                ptp[:, u * 512:(u + 1) * 512],
                                start=(kt == 0), stop=(kt == 2 * npair - 1),
                            )

                    for pj in range(npair):
                        sp = psp.tile([128, 1024], f32, tag="ps")
                        for u in range(2):
                            kt = 2 * pj + u
                            nc.tensor.matmul(
                                sp[:, u * 512:(u + 1) * 512],
                                kt_tb[kt // 4][:, g, (kt % 4) * 128:(kt % 4 + 1) * 128],
                                qt_tb[qb][:, h, :],
                                start=True, stop=True,
                            )
                        if pj >= npair - 2:
                            for u in range(2):
                                kt_loc = 2 * (pj - (npair - 2)) + u
                                off = 128 * kt_loc
                                w = min(512, 128 + off)
                                s0 = 384 - off
                                nc.vector.tensor_tensor(
                                    sp[:, u * 512:u * 512 + w], sp[:, u * 512:u * 512 + w],
                                    mask2[:, s0:s0 + w],
                                    mybir.AluOpType.add,
                                )
                        ptp = ptpool.tile([128, 1024], bf16, tag="pt")
                        nc.scalar.activation(ptp[:], sp[:], EXP)
                        pend.append((pj, ptp))
                        if len(pend) >= 3:
                            flush_pv()
                    while pend:
                        flush_pv()

                    # deferred normalize of the previous head: its GpSimd
                    # broadcast must queue after this head's masks so an
                    # in-order GpSimd never blocks masks behind it
                    if pending_tails:
                        normalize_tail(*pending_tails.pop(0))
                    pending_tails.append((cp, h, qb))

                    # interleave previous q-block's output projection
                    if qb >= 1 and hidx < 8:
                        tt = (qb - 1) * 4 + hidx // 2
                        ob = (hidx % 2) * 2
                        outproj_tile(tt, ob)
                        outproj_tile(tt, ob + 1)

            for t_ in pending_tails:
                normalize_tail(*t_)

            if _DEBUG:
                nc.sync.dma_start(d_dq.ap()[:], qt_tb[0][:])
                nc.sync.dma_start(d_dk.ap()[:], kt_tb[0][:])
                nc.sync.dma_start(d_dv.ap()[:], va_tb[0][:])
                nc.sync.dma_start(d_dc.ap()[:], cx_qb[0][:])

            for hidx in range(8):
                tt = 12 + hidx // 2
                ob = (hidx % 2) * 2
                outproj_tile(tt, ob)
                outproj_tile(tt, ob + 1)

    nc.compile()
    return nc


def _get_nc():
    global _NC
    if _NC is None:
        _NC = _build()
    return _NC


def _prep_inputs(x, Wq, Wk, Wv, Wo):
    bf = ml_dtypes.bfloat16
    xt_b = []
    for b in range(B):
        xt = np.ascontiguousarray(x[b].T).astype(bf)  # [D, S]
        xt_b.append(xt.reshape(16, 128, S))
    # master causal additive mask: valid iff p <= j - 384
    p = np.arange(128)[:, None]
    j = np.arange(896)[None, :]
    mask2 = np.where(p <= j - 384, 0.0, -1e30).astype(np.float32)  # [128, 896]

    in_maps = []
    for c in range(8):
        b, tp = divmod(c, 4)
        wq_s = (0.125 * Wq[tp * 512:(tp + 1) * 512]).astype(np.float32)  # [512, D]
        wqt = np.ascontiguousarray(wq_s.T).astype(bf).reshape(16, 128, 512)
        wkt = np.ascontiguousarray(Wk[tp * 128:(tp + 1) * 128].T).astype(bf).reshape(16, 128, 128)
        wvt = np.ascontiguousarray(Wv[tp * 128:(tp + 1) * 128].T).astype(bf).reshape(16, 128, 128)
        wot = np.ascontiguousarray(Wo[:, tp * 512:(tp + 1) * 512].T).astype(bf).reshape(4, 128, D)
        in_maps.append({
            "xt": xt_b[b], "wqt": wqt, "wkt": wkt, "wvt": wvt, "wot": wot,
            "mask2": mask2,
        })
    return in_maps


def _run(x, Wq, Wk, Wv, Wo, trace=False, trace_cores=None):
    from concourse.bass_utils import run_bass_kernel_spmd

    nc = _get_nc()
    in_maps = _prep_inputs(x, Wq, Wk, Wv, Wo)
    kwargs = {}
    if trace:
        kwargs = dict(trace=True, trace_cores=trace_cores or [0])
    res = run_bass_kernel_spmd(nc, in_maps, core_ids=list(range(8)), **kwargs)

    out = np.zeros((B, S, D), dtype=np.float32)
    keys = np.zeros((B, 8, S, HD), dtype=np.float32)
    values = np.zeros((B, 8, S, HD), dtype=np.float32)
    for c in range(8):
        b, tp = divmod(c, 4)
        r = res.results[c]
        out[b] += r["out"]
        for g in range(2):
            keys[b, 2 * tp + g] = r["kc"][g].T
            values[b, 2 * tp + g] = r["vc"][:, g * 64:(g + 1) * 64]
    return (out, (keys, values)), res


def kernel(x, Wq, Wk, Wv, Wo):
    outputs, _ = _run(np.asarray(x), np.asarray(Wq), np.asarray(Wk),
                      np.asarray(Wv), np.asarray(Wo))
    return outputs


# revision 18
# speedup vs baseline: 1.2096x; 1.0391x over previous
"""GroupedQueryAttention Trainium2 Bass kernel.

Problem: B=2, S=2048, D_IN=2048, 32 Q heads / 8 KV groups, head_dim=64.
Sharding: data-parallel over batch (2) x tensor-parallel over KV groups (4):
core c -> batch c//4, group-pair c%4 (2 KV groups = 8 Q heads per core).
Each core computes its heads' attention plus a partial output projection;
the host sums the 4 partial outputs per batch.

Per-core kernel design (all matmuls bf16 with fp32 PSUM accumulation):
 - Host passes x^T and transposed weight slices so every matmul contracts
   over the partition dim with no on-device transposes of x or W.
 - Scores are computed transposed, S^T[k,q], so softmax exp() output IS
   P^T (the layout the PV matmul needs) - no P transposes on any engine.
 - No max-subtraction in softmax (scores are O(3), exp is safe in fp32;
   mathematically identical to the reference softmax).
 - The softmax denominator rides along as a 65th "ones" column of V:
   ctx_aug^T = V_aug^T @ P^T gives ctx^T rows 0..63 and the denominator
   in row 64 of the same PSUM accumulation.
 - kt blocks processed in pairs sharing one [128,1024] PSUM tile and one
   exp() to amortize the ACT fixed overhead; causal mask = bf16 0/1
   multiply on GpSimd for the two diagonal pairs only.
 - Per-tb/per-qb tiles keep cross-phase dependencies block-granular; the
   output projection for q-block qb is emitted inside attention of qb+1
   to fill TensorE gaps while ACT runs exp.
"""

import numpy as np
import ml_dtypes

B, S, D = 2, 2048, 2048
HD = 64

_NC = None
_DEBUG = False


def _build():
    import concourse.mybir as mybir
    import concourse.tile as tile
    from concourse import bacc

    bf16 = mybir.dt.bfloat16
    f32 = mybir.dt.float32
    EXP = mybir.ActivationFunctionType.Exp
    MUL = mybir.AluOpType.mult

    nc = bacc.Bacc("TRN2", target_bir_lowering=False, debug=False, num_devices=8)

    d_xt = nc.dram_tensor("xt", [16, 128, S], bf16, kind="ExternalInput")
    d_wqt = nc.dram_tensor("wqt", [16, 128, 512], bf16, kind="ExternalInput")
    d_wkt = nc.dram_tensor("wkt", [16, 128, 128], bf16, kind="ExternalInput")
    d_wvt = nc.dram_tensor("wvt", [16, 128, 128], bf16, kind="ExternalInput")
    d_wot = nc.dram_tensor("wot", [4, 128, D], bf16, kind="ExternalInput")
    d_mask2 = nc.dram_tensor("mask2", [2, 128, 1024], f32, kind="ExternalInput")

    d_out = nc.dram_tensor("out", [S, D], f32, kind="ExternalOutput")
    if _DEBUG:
        d_dq = nc.dram_tensor("dq", [64, 8, 512], bf16, kind="ExternalOutput")
        d_dk = nc.dram_tensor("dk", [64, 2, 512], bf16, kind="ExternalOutput")
        d_dv = nc.dram_tensor("dv", [128, 4, 2, 65], bf16, kind="ExternalOutput")
        d_dc = nc.dram_tensor("dc", [128, 4, 512], bf16, kind="ExternalOutput")
    d_kc = nc.dram_tensor("kc", [2, 64, S], f32, kind="ExternalOutput")
    d_vc = nc.dram_tensor("vc", [S, 128], f32, kind="ExternalOutput")

    with tile.TileContext(nc) as tc:
        with (
            tc.tile_pool(name="cst", bufs=1) as cst,
            tc.tile_pool(name="xtp", bufs=32) as xtp,
            tc.tile_pool(name="wk", bufs=4) as wk,
            tc.tile_pool(name="ptp", bufs=6) as ptpool,
            tc.tile_pool(name="ps", bufs=2, space="PSUM") as psp,
            tc.tile_pool(name="psc", bufs=2, space="PSUM") as psc,
            tc.tile_pool(name="pso", bufs=2, space="PSUM") as pso,
        ):
            # weights needed first go first; wot (phase 3 only) last
            wkt = cst.tile([128, 16, 128], bf16, tag="wkt")
            nc.sync.dma_start(wkt[:], d_wkt.ap().rearrange("i p f -> p i f"))
            wvt = cst.tile([128, 16, 128], bf16, tag="wvt")
            nc.sync.dma_start(wvt[:], d_wvt.ap().rearrange("i p f -> p i f"))
            wqt = cst.tile([128, 16, 512], bf16, tag="wqt")
            nc.sync.dma_start(wqt[:], d_wqt.ap().rearrange("i p f -> p i f"))
            mask2 = cst.tile([128, 2, 1024], f32, tag="mask2")
            wot = cst.tile([128, 4, D], bf16, tag="wot")

            # per-block persistent activations
            qt_tb = [cst.tile([64, 8, 512], bf16, tag=f"qt{tb}", name=f"qt{tb}") for tb in range(4)]
            kt_tb = [cst.tile([64, 2, 512], bf16, tag=f"kt{tb}", name=f"kt{tb}") for tb in range(4)]
            va_tb = [cst.tile([128, 4, 2, 65], bf16, tag=f"va{tb}", name=f"va{tb}") for tb in range(4)]
            cx_qb = [cst.tile([128, 4, 512], bf16, tag=f"cx{qb}", name=f"cx{qb}") for qb in range(4)]
            for tb in range(4):
                nc.vector.memset(va_tb[tb][:, :, :, 64:65], 1.0)

            # ---------------- Phase 1: projections ----------------
            for tb in range(4):
                if tb == 1:
                    # deferred input DMAs: not needed until attention/outproj
                    nc.sync.dma_start(mask2[:], d_mask2.ap().rearrange("m p j -> p m j"))
                    nc.sync.dma_start(wot[:], d_wot.ap().rearrange("c p o -> p c o"))
                ts_ = slice(tb * 512, (tb + 1) * 512)
                xts = []
                for it in range(16):
                    xt = xtp.tile([128, 512], bf16, tag="xt")
                    nc.sync.dma_start(xt[:], d_xt.ap()[it, :, ts_])
                    xts.append(xt)

                # K projection: both groups in one 128-wide output
                kp = psp.tile([128, 1024], f32, tag="ps")
                for it in range(16):
                    nc.tensor.matmul(
                        kp[:, :512], wkt[:, it, :], xts[it][:],
                        start=(it == 0), stop=(it == 15),
                    )
                kstg = wk.tile([128, 512], bf16, tag="kstg")
                nc.any.tensor_copy(out=kstg[:], in_=kp[:, :512])
                for g in range(2):
                    nc.sync.dma_start(
                        kt_tb[tb][:, g, :], kstg[g * 64:(g + 1) * 64, :]
                    )
                kf = wk.tile([128, 512], f32, tag="kf")
                nc.any.tensor_copy(out=kf[:], in_=kp[:, :512])
                for g in range(2):
                    nc.sync.dma_start(
                        d_kc.ap()[g, :, ts_], kf[g * 64:(g + 1) * 64, :]
                    )

                # Q projection: head pairs, full 128-wide outputs
                for pj in range(4):
                    qp = psp.tile([128, 1024], f32, tag="ps")
                    for it in range(16):
                        nc.tensor.matmul(
                            qp[:, :512], wqt[:, it, pj * 128:(pj + 1) * 128],
                            xts[it][:], start=(it == 0), stop=(it == 15),
                        )
                    qstg = wk.tile([128, 512], bf16, tag="qstg")
                    nc.any.tensor_copy(out=qstg[:], in_=qp[:, :512])
                    for j in range(2):
                        nc.sync.dma_start(
                            qt_tb[tb][:, 2 * pj + j, :],
                            qstg[j * 64:(j + 1) * 64, :],
                        )

                # V projection: token-major tiles
                for tl in range(4):
                    vp = psp.tile([128, 1024], f32, tag="ps")
                    for it in range(16):
                        nc.tensor.matmul(
                            vp[:, :128], xts[it][:, tl * 128:(tl + 1) * 128],
                            wvt[:, it, :], start=(it == 0), stop=(it == 15),
                        )
                    for g in range(2):
                        nc.any.tensor_copy(
                            out=va_tb[tb][:, tl, g, 0:64],
                            in_=vp[:, g * 64:(g + 1) * 64],
                        )
                    vf = wk.tile([128, 128], f32, tag="vf")
                    nc.any.tensor_copy(out=vf[:], in_=vp[:, :128])
                    tt = tb * 4 + tl
                    nc.sync.dma_start(d_vc.ap()[tt * 128:(tt + 1) * 128, :], vf[:])

            # ------------- Phase 2 + interleaved output projection -------------
            def outproj_tile(tt, ob):
                qb = tt // 4
                tsl = slice((tt % 4) * 128, (tt % 4 + 1) * 128)
                osl = slice(ob * 512, (ob + 1) * 512)
                op = pso.tile([128, 512], f32, tag="pso")
                for ct in range(4):
                    nc.tensor.matmul(
                        op[:], cx_qb[qb][:, ct, tsl], wot[:, ct, osl],
                        start=(ct == 0), stop=(ct == 3),
                    )
                ot = wk.tile([128, 512], f32, tag="ot")
                nc.vector.tensor_copy(out=ot[:], in_=op[:])
                tgl = slice((tt) * 128, (tt + 1) * 128)
                nc.sync.dma_start(d_out.ap()[tgl, osl], ot[:])

            pending_tails = []

            def normalize_tail(cp, h, qb):
                # row 64 of cp is the softmax denominator
                rr = wk.tile([1, 512], f32, tag="rr", name="rr")
                nc.vector.reciprocal(rr[:], cp[64:65, :])
                rbs = wk.tile([64, 512], f32, tag="rbs", name="rbs")
                nc.gpsimd.partition_broadcast(rbs[:], rr[:])
                stg = wk.tile([64, 512], bf16, tag="stg", name="stg")
                nc.vector.tensor_tensor(stg[:], cp[0:64, :], rbs[:], MUL)
                poff = (h % 2) * 64
                nc.sync.dma_start(cx_qb[qb][poff:poff + 64, h // 2, :], stg[:])

            for qb in range(4):
                npair = 2 * (qb + 1)
                for hidx in range(8):
                    g, r4 = divmod(hidx, 4)
                    h = hidx
                    cp = psc.tile([65, 512], f32, tag="psc")
                    pend = []

                    def flush_pv(cp=cp, npair=npair, g=g, pend=pend):
                        pj, ptp = pend.pop(0)
                        for u in range(2):
                            kt = 2 * pj + u
                            nc.tensor.matmul(
                                cp[:], va_tb[kt // 4][:, kt % 4, g, :],
                                ptp[:, u * 512:(u + 1) * 512],
                                start=(kt == 0), stop=(kt == 2 * npair - 1),
                            )

                    for pj in range(npair):
                        sp = psp.tile([128, 1024], f32, tag="ps")
                        for u in range(2):
                            kt = 2 * pj + u
                            nc.tensor.matmul(
                                sp[:, u * 512:(u + 1) * 512],
                                kt_tb[kt // 4][:, g, (kt % 4) * 128:(kt % 4 + 1) * 128],
                                qt_tb[qb][:, h, :],
                                start=True, stop=True,
                            )
                        if pj >= npair - 2:
                            pl = pj - (npair - 2)
                            nc.vector.tensor_tensor(
                                sp[:], sp[:], mask2[:, pl, :],
                                mybir.AluOpType.add,
                            )
                        ptp = ptpool.tile([128, 1024], bf16, tag="pt")
                        nc.scalar.activation(ptp[:], sp[:], EXP)
                        pend.append((pj, ptp))
                        if len(pend) >= 3:
                            flush_pv()
                    while pend:
                        flush_pv()

                    # deferred normalize of the previous head: its GpSimd
                    # broadcast must queue after this head's masks so an
                    # in-order GpSimd never blocks masks behind it
                    if pending_tails:
                        normalize_tail(*pending_tails.pop(0))
                    pending_tails.append((cp, h, qb))

                    # interleave previous q-block's output projection
                    if qb >= 1 and hidx < 8:
                        tt = (qb - 1) * 4 + hidx // 2
                        ob = (hidx % 2) * 2
                        outproj_tile(tt, ob)
                        outproj_tile(tt, ob + 1)

            for t_ in pending_tails:
                normalize_tail(*t_)

            if _DEBUG:
                nc.sync.dma_start(d_dq.ap()[:], qt_tb[0][:])
                nc.sync.dma_start(d_dk.ap()[:], kt_tb[0][:])
                nc.sync.dma_start(d_dv.ap()[:], va_tb[0][:])
                nc.sync.dma_start(d_dc.ap()[:], cx_qb[0][:])

            for hidx in range(8):
                tt = 12 + hidx // 2
                ob = (hidx % 2) * 2
                outproj_tile(tt, ob)
                outproj_tile(tt, ob + 1)

    nc.compile()
    return nc


def _get_nc():
    global _NC
    if _NC is None:
        _NC = _build()
    return _NC


def _prep_inputs(x, Wq, Wk, Wv, Wo):
    bf = ml_dtypes.bfloat16
    xt_b = []
    for b in range(B):
        xt = np.ascontiguousarray(x[b].T).astype(bf)  # [D, S]
        xt_b.append(xt.reshape(16, 128, S))
    # causal pair masks: halves for kt_loc (0,1) and (2,3)
    p = np.arange(128)[:, None]
    j = np.arange(512)[None, :]

    def m(off):
        return np.where(p + off <= j, 0.0, -1e30).astype(np.float32)

    mask2 = np.stack([
        np.concatenate([m(0), m(128)], axis=1),
        np.concatenate([m(256), m(384)], axis=1),
    ])  # [2, 128, 1024] f32 additive

    in_maps = []
    for c in range(8):
        b, tp = divmod(c, 4)
        wq_s = (0.125 * Wq[tp * 512:(tp + 1) * 512]).astype(np.float32)  # [512, D]
        wqt = np.ascontiguousarray(wq_s.T).astype(bf).reshape(16, 128, 512)
        wkt = np.ascontiguousarray(Wk[tp * 128:(tp + 1) * 128].T).astype(bf).reshape(16, 128, 128)
        wvt = np.ascontiguousarray(Wv[tp * 128:(tp + 1) * 128].T).astype(bf).reshape(16, 128, 128)
        wot = np.ascontiguousarray(Wo[:, tp * 512:(tp + 1) * 512].T).astype(bf).reshape(4, 128, D)
        in_maps.append({
            "xt": xt_b[b], "wqt": wqt, "wkt": wkt, "wvt": wvt, "wot": wot,
            "mask2": mask2,
        })
    return in_maps


def _run(x, Wq, Wk, Wv, Wo, trace=False, trace_cores=None):
    from concourse.bass_utils import run_bass_kernel_spmd

    nc = _get_nc()
    in_maps = _prep_inputs(x, Wq, Wk, Wv, Wo)
    kwargs = {}
    if trace:
        kwargs = dict(trace=True, trace_cores=trace_cores or [0])
    res = run_bass_kernel_spmd(nc, in_maps, core_ids=list(range(8)), **kwargs)

    out = np.zeros((B, S, D), dtype=np.float32)
    keys = np.zeros((B, 8, S, HD), dtype=np.float32)
    values = np.zeros((B, 8, S, HD), dtype=np.float32)
    for c in range(8):
        b, tp = divmod(c, 4)
        r = res.results[c]
        out[b] += r["out"]
        for g in range(2):
            keys[b, 2 * tp + g] = r["kc"][g].T
            values[b, 2 * tp + g] = r["vc"][:, g * 64:(g + 1) * 64]
    return (out, (keys, values)), res


def kernel(x, Wq, Wk, Wv, Wo):
    outputs, _ = _run(np.asarray(x), np.asarray(Wq), np.asarray(Wk),
                      np.asarray(Wv), np.asarray(Wo))
    return outputs
